# revision 29
# baseline (speedup 1.0000x reference)
"""GRU (hard-sigmoid gates, tanh candidate) Trainium2 kernel, 8 NeuronCores.

Strategy:
  - Data-parallel: batch 32 -> 4 per core. The T=512 recurrence is strictly
    sequential, and collectives have a ~5us floor, so each core runs its own
    batch shard's full recurrence locally (no cross-core traffic).
  - Everything lives transposed: h^T [U_part, B_free], mx^T [3U_part, T, B].
    The recurrent matmul uses the native recurrent_kernel [U, 3U] layout as
    the stationary operand (lhsT), streaming h^T [128, 4] as the moving
    operand -> output lands already transposed, elementwise ops use all 128
    partitions, and no per-step transposes are needed.
  - bf16 weights/h (fp32 matmul is 4 cycles/row; bf16 is 1), f32 PSUM.
  - hard_sigmoid folding: z/r columns of both weight matrices are pre-scaled
    by 0.2 on the host and mx for those columns gets bias' = 0.2*bias + 0.5,
    so z|r = clip(psum, 0, 1) directly.
  - The mx additive term is folded into PSUM by an identity matmul that
    initializes each accumulation group (start=True), so no DVE adds are on
    the critical path; clip and tanh read PSUM directly.
  - Blend h' = z*h + (1-z)*hh is two post-tanh DVE ops: a=z*h and w=1-z are
    precomputed in the hh-matmul shadow; then f=w*hh, h'=f+a.
  - h' is written directly into a persistent bf16 history buffer (slot s+1);
    the next step's matmuls read slot s. History bulk-DMAs to DRAM (bf16)
    every 64 steps; host upcasts to f32.
  - Outer For_i hardware loop (8 iters) x 64 python-unrolled steps with
    ping-pong prefetch of the staged mx blocks.
"""

import os
import sys
from contextlib import ExitStack

sys.path.insert(0, "/opt/trn_rl_repo")

import numpy as np
import ml_dtypes

import concourse.bass as bass
import concourse.tile as tile
from concourse import bacc, mybir
from concourse.bass_utils import run_bass_kernel_spmd
from concourse.masks import make_identity
from concourse.tile_autobufs import add_dep_helper


def _install_ntff_hook():
    """The container's antenv stub lacks axon_hooks; provide it so
    trace=True (used by test.py for profiling) works. No-op on failure."""
    import types

    try:
        import antenv
        if "antenv.axon_hooks" in sys.modules:
            return
        mod = types.ModuleType("antenv.axon_hooks")
        state = {"h": None}
        mod.set_axon_ntff_profile_hook = lambda h: state.__setitem__("h", h)
        mod.get_axon_ntff_profile_hook = lambda: state["h"]
        sys.modules["antenv.axon_hooks"] = mod
        antenv.axon_hooks = mod
        from trn_agent_boot.trn_boot import _ntff_profile_via_ctypes
        mod.set_axon_ntff_profile_hook(
            _ntff_profile_via_ctypes("/opt/axon/libaxon_pjrt.so")
        )
    except Exception:
        pass


_install_ntff_hook()

B, T, D, U = 32, 512, 512, 512
NCORES = 8
BL = B // NCORES          # 4 batches per core
KC = D // 128             # 4 contraction chunks (input proj)
UC = U // 128             # 4 contraction chunks (recurrent)
M_ALL = 3 * U // 128      # 12 output column chunks
SBLK = 32                 # steps per staged mx block
BODY = 2 * SBLK           # steps per For_i body (ping-pong A/B)

BF16 = mybir.dt.bfloat16
F32 = mybir.dt.float32
Alu = mybir.AluOpType
Act = mybir.ActivationFunctionType
ET = mybir.EngineType

_CACHE = {}
LAST_RESULT = None


def _build(T=T):
    nc = bacc.Bacc()
    xT = nc.declare_dram_parameter("xT", [D, BL * T], BF16, isOutput=False)
    wk = nc.declare_dram_parameter("wk", [D, 3 * U], BF16, isOutput=False)
    wr = nc.declare_dram_parameter("wr", [U, 3 * U], BF16, isOutput=False)
    bp = nc.declare_dram_parameter("bp", [3 * U], F32, isOutput=False)
    # out[u%128, u//128, t, b] (bf16; host upcasts)
    out = nc.declare_dram_parameter("out", [128, UC, T, BL], BF16, isOutput=True)

    with tile.TileContext(nc) as tc, ExitStack() as ctx:
        consts = ctx.enter_context(tc.tile_pool(name="consts", bufs=1))
        psum_p = ctx.enter_context(tc.tile_pool(name="psum", bufs=2, space="PSUM"))
        psum_1 = ctx.enter_context(tc.tile_pool(name="psum1", bufs=1, space="PSUM"))
        work = ctx.enter_context(tc.tile_pool(name="work", bufs=2))

        wk_sb = consts.tile([128, KC, 3 * U], BF16)
        nc.sync.dma_start(out=wk_sb, in_=wk.rearrange("(c p) n -> p c n", p=128))
        wr_sb = consts.tile([128, UC, 3 * U], BF16)
        nc.sync.dma_start(out=wr_sb, in_=wr.rearrange("(c p) n -> p c n", p=128))
        bp_sb = consts.tile([128, M_ALL], F32)
        nc.sync.dma_start(out=bp_sb, in_=bp.rearrange("(m p) -> p m", p=128))
        xT_sb = consts.tile([128, KC, BL * T], BF16)
        nc.sync.dma_start(out=xT_sb, in_=xT.rearrange("(c p) n -> p c n", p=128))
        ident = consts.tile([128, 128], BF16)
        make_identity(nc, ident)

        # mx^T [n%128, n//128, t, b] bf16, padded by BODY junk steps so the
        # ping-pong prefetch can always read a full block
        mx_sb = consts.tile([128, M_ALL, T + BODY, BL], BF16)
        nc.vector.memset(mx_sb[:, :, T:, :], 0.0)

        # ---- phase 1: mx^T = kernel^T @ x^T (+ bias', x0.2 pre-folded) ----
        for b in range(BL):  # xT columns are (b, t): one 512-col tile per batch
            for m in range(M_ALL):
                ps = psum_1.tile([128, T], F32, tag="p1")
                for d in range(KC):
                    nc.tensor.matmul(
                        ps,
                        lhsT=wk_sb[:, d, m * 128:(m + 1) * 128],
                        rhs=xT_sb[:, d, b * T:(b + 1) * T],
                        start=(d == 0),
                        stop=(d == KC - 1),
                    )
                nc.scalar.activation(
                    out=mx_sb[:, m, 0:T, b], in_=ps, func=Act.Identity,
                    bias=bp_sb[:, m:m + 1],
                )

        # ---- phase 2: recurrence ----
        # persistent bf16 history: step s reads slot s, writes slot s+1;
        # the last step wraps to slot 0 (becomes next body's h_in) so no
        # carry copy is needed.
        hist = consts.tile([128, UC, BODY, BL], BF16)
        nc.vector.memset(hist[:, :, 0:1, :], 0.0)
        stgA = consts.tile([128, M_ALL, SBLK, BL], BF16)
        stgB = consts.tile([128, M_ALL, SBLK, BL], BF16)
        nc.sync.dma_start(out=stgA, in_=mx_sb[:, :, 0:SBLK, :])

        def step(stg, s, slot):
            out_slot = (slot + 1) % BODY
            h_in = hist[:, :, slot, :]                    # [128, UC, BL] bf16
            h_in4 = hist[:, :, slot:slot + 1, :]          # [128, UC, 1, BL]
            # Separate PSUM tiles for r / z / hh-halves so each consumer
            # depends only on its own accumulation group. One group per tile:
            # start on its first id-MM, stop on its last weight-MM. All id-MMs
            # (mx init, no h dependency) are emitted first so the PE runs them
            # during the previous step's blend.
            pr = psum_p.tile([128, 4, 1, BL], F32, tag="r")
            pzz = psum_1.tile([128, 4, 1, BL], F32, tag="z")
            phA = psum_p.tile([128, 2, 1, BL], F32, tag="hhA")
            phB = psum_p.tile([128, 2, 1, BL], F32, tag="hhB")
            for m in range(4):
                nc.tensor.matmul(
                    pr[:, m, 0, :], lhsT=ident, rhs=stg[:, 4 + m, s, :],
                    start=(m == 0), stop=False, skip_group_check=True,
                )
            for m in range(4):
                nc.tensor.matmul(
                    pzz[:, m, 0, :], lhsT=ident, rhs=stg[:, m, s, :],
                    start=(m == 0), stop=False, skip_group_check=True,
                )
            for m in range(2):
                nc.tensor.matmul(
                    phA[:, m, 0, :], lhsT=ident, rhs=stg[:, 8 + m, s, :],
                    start=(m == 0), stop=False, skip_group_check=True,
                )
            for m in range(2):
                nc.tensor.matmul(
                    phB[:, m, 0, :], lhsT=ident, rhs=stg[:, 10 + m, s, :],
                    start=(m == 0), stop=False, skip_group_check=True,
                )
            # r-gate weight MMs first, k-outer so the k=0,1 MMs only need the
            # first half of the blended h (chunked handoff from prev step)
            for k in range(UC):
                for m in range(4):
                    nc.tensor.matmul(
                        pr[:, m, 0, :],
                        lhsT=wr_sb[:, k, (4 + m) * 128:(5 + m) * 128],
                        rhs=h_in[:, k, :],
                        start=False,
                        stop=(k == UC - 1 and m == 3),
                        skip_group_check=True,
                    )
            for k in range(UC):
                for m in range(4):
                    nc.tensor.matmul(
                        pzz[:, m, 0, :],
                        lhsT=wr_sb[:, k, m * 128:(m + 1) * 128],
                        rhs=h_in[:, k, :],
                        start=False,
                        stop=(k == UC - 1 and m == 3),
                        skip_group_check=True,
                    )
            # r = clip(psum_r, 0, 1); rh = r * h   (unblocks hh matmuls)
            r_bf = work.tile([128, 4, 1, BL], BF16, tag="rbf")
            nc.vector.tensor_scalar(r_bf, pr, 1.0, 0.0,
                                    op0=Alu.min, op1=Alu.max)
            rh = work.tile([128, UC, 1, BL], BF16, tag="rh")
            rh_i = nc.vector.tensor_mul(rh, r_bf, h_in4)
            # hh pre-activation: psum = mx_h' + rh @ W_h; m-halves so tanh_A
            # can run while the B-half matmuls still execute
            for m in range(2):
                for k in range(UC):
                    nc.tensor.matmul(
                        phA[:, m, 0, :],
                        lhsT=wr_sb[:, k, 2 * U + m * 128:2 * U + (m + 1) * 128],
                        rhs=rh[:, k, 0, :],
                        start=False,
                        stop=(m == 1 and k == UC - 1),
                        skip_group_check=True,
                    )
            for m in range(2, 4):
                for k in range(UC):
                    nc.tensor.matmul(
                        phB[:, m - 2, 0, :],
                        lhsT=wr_sb[:, k, 2 * U + m * 128:2 * U + (m + 1) * 128],
                        rhs=rh[:, k, 0, :],
                        start=False,
                        stop=(m == 3 and k == UC - 1),
                        skip_group_check=True,
                    )
            # z ops off the critical chain (clip_z ordered after rh)
            z_bf = work.tile([128, 4, 1, BL], BF16, tag="zbf")
            zb_i = nc.vector.tensor_scalar(z_bf, pzz, 1.0, 0.0,
                                           op0=Alu.min, op1=Alu.max)
            add_dep_helper(zb_i.ins, rh_i.ins, sync=False,
                           reason="DVE critical chain first")
            w_t = work.tile([128, 4, 1, BL], F32, tag="wt")
            nc.vector.tensor_scalar(w_t, z_bf, -1.0, 1.0,
                                    op0=Alu.mult, op1=Alu.add)      # 1-z
            a_t = work.tile([128, 4, 1, BL], F32, tag="at")
            nc.vector.tensor_mul(a_t, z_bf, h_in4)
            # hh = tanh(psum); h' = (1-z)*hh + z*h, in halves -> hist out_slot
            hh_A = work.tile([128, 2, 1, BL], F32, tag="hhA2")
            nc.scalar.activation(out=hh_A, in_=phA, func=Act.Tanh)
            f_A = work.tile([128, 2, 1, BL], F32, tag="ftA")
            nc.vector.tensor_mul(f_A, w_t[:, 0:2, :, :], hh_A)
            nc.vector.tensor_add(hist[:, 0:2, out_slot:out_slot + 1, :],
                                 f_A, a_t[:, 0:2, :, :])
            hh_B = work.tile([128, 2, 1, BL], F32, tag="hhB2")
            nc.scalar.activation(out=hh_B, in_=phB, func=Act.Tanh)
            f_B = work.tile([128, 2, 1, BL], F32, tag="ftB")
            nc.vector.tensor_mul(f_B, w_t[:, 2:4, :, :], hh_B)
            nc.vector.tensor_add(hist[:, 2:4, out_slot:out_slot + 1, :],
                                 f_B, a_t[:, 2:4, :, :])

        with tc.For_i(0, T, BODY,
                      hint_engines=(ET.PE, ET.DVE, ET.Activation,
                                    ET.SP, ET.Pool)) as i:
            nc.sync.dma_start(out=stgB,
                              in_=mx_sb[:, :, bass.ds(i + SBLK, SBLK), :])
            for s in range(SBLK):
                step(stgA, s, s)
            nc.sync.dma_start(out=stgA,
                              in_=mx_sb[:, :, bass.ds(i + BODY, SBLK), :])
            for s in range(SBLK):
                step(stgB, s, SBLK + s)
            nc.sync.dma_start(out=out[:, :, bass.ds(i, BODY - 1), :],
                              in_=hist[:, :, 1:BODY, :])
            nc.sync.dma_start(out=out[:, :, bass.ds(i + BODY - 1, 1), :],
                              in_=hist[:, :, 0:1, :])
    return nc


def _graph():
    if "nc" not in _CACHE:
        nc = _build()
        if not nc.is_finalized():
            nc.finalize()
        _CACHE["nc"] = nc
    return _CACHE["nc"]


def kernel(x, kernel, recurrent_kernel, bias):
    global LAST_RESULT
    x = np.asarray(x, dtype=np.float32)
    wk_f = np.asarray(kernel, dtype=np.float32)
    wr_f = np.asarray(recurrent_kernel, dtype=np.float32)
    b_f = np.asarray(bias, dtype=np.float32)

    # fold hard_sigmoid affine (0.2*x + 0.5) into the z|r weight columns/bias
    scale = np.ones((3 * U,), np.float32)
    scale[: 2 * U] = 0.2
    wk_h = (wk_f * scale).astype(ml_dtypes.bfloat16)
    wr_h = (wr_f * scale).astype(ml_dtypes.bfloat16)
    bp_h = np.where(np.arange(3 * U) < 2 * U, 0.2 * b_f + 0.5, b_f).astype(np.float32)

    in_maps = []
    for c in range(NCORES):
        xs = x[c * BL:(c + 1) * BL]                       # [BL, T, D]
        xTc = np.ascontiguousarray(
            xs.transpose(2, 0, 1).reshape(D, BL * T)
        ).astype(ml_dtypes.bfloat16)
        in_maps.append({"xT": xTc, "wk": wk_h, "wr": wr_h, "bp": bp_h})

    res = run_bass_kernel_spmd(
        _graph(), in_maps, core_ids=list(range(NCORES)),
        trace=bool(os.environ.get("GRU_TRACE")),
    )
    LAST_RESULT = res

    outs = []
    for c in range(NCORES):
        arr = np.asarray(res.results[c]["out"]).astype(np.float32)
        outs.append(np.transpose(arr, (3, 2, 1, 0)).reshape(BL, T, U))
    return np.concatenate(outs, axis=0)


# revision 31
# speedup vs baseline: 1.0680x; 1.0680x over previous
"""GRU (hard-sigmoid gates, tanh candidate) Trainium2 kernel, 8 NeuronCores.

Strategy:
  - Data-parallel: batch 32 -> 4 per core. The T=512 recurrence is strictly
    sequential, and collectives have a ~5us floor, so each core runs its own
    batch shard's full recurrence locally (no cross-core traffic).
  - Everything lives transposed: h^T [U_part, B_free], mx^T [3U_part, T, B].
    The recurrent matmul uses the native recurrent_kernel [U, 3U] layout as
    the stationary operand (lhsT), streaming h^T [128, 4] as the moving
    operand -> output lands already transposed, elementwise ops use all 128
    partitions, and no per-step transposes are needed.
  - bf16 weights/h (fp32 matmul is 4 cycles/row; bf16 is 1), f32 PSUM.
  - hard_sigmoid folding: z/r columns of both weight matrices are pre-scaled
    by 0.2 on the host and mx for those columns gets bias' = 0.2*bias + 0.5,
    so z|r = clip(psum, 0, 1) directly.
  - The mx additive term is folded into PSUM by an identity matmul that
    initializes each accumulation group (start=True), so no DVE adds are on
    the critical path; clip and tanh read PSUM directly.
  - Blend h' = z*h + (1-z)*hh is two post-tanh DVE ops: a=z*h and w=1-z are
    precomputed in the hh-matmul shadow; then f=w*hh, h'=f+a.
  - h' is written directly into a persistent bf16 history buffer (slot s+1);
    the next step's matmuls read slot s. History bulk-DMAs to DRAM (bf16)
    every 64 steps; host upcasts to f32.
  - Outer For_i hardware loop (8 iters) x 64 python-unrolled steps with
    ping-pong prefetch of the staged mx blocks.
"""

import os
import sys
from contextlib import ExitStack

sys.path.insert(0, "/opt/trn_rl_repo")

import numpy as np
import ml_dtypes

import concourse.bass as bass
import concourse.tile as tile
from concourse import bacc, mybir
from concourse.bass_utils import run_bass_kernel_spmd
from concourse.masks import make_identity
from concourse.tile_autobufs import add_dep_helper


def _install_ntff_hook():
    """The container's antenv stub lacks axon_hooks; provide it so
    trace=True (used by test.py for profiling) works. No-op on failure."""
    import types

    try:
        import antenv
        if "antenv.axon_hooks" in sys.modules:
            return
        mod = types.ModuleType("antenv.axon_hooks")
        state = {"h": None}
        mod.set_axon_ntff_profile_hook = lambda h: state.__setitem__("h", h)
        mod.get_axon_ntff_profile_hook = lambda: state["h"]
        sys.modules["antenv.axon_hooks"] = mod
        antenv.axon_hooks = mod
        from trn_agent_boot.trn_boot import _ntff_profile_via_ctypes
        mod.set_axon_ntff_profile_hook(
            _ntff_profile_via_ctypes("/opt/axon/libaxon_pjrt.so")
        )
    except Exception:
        pass


_install_ntff_hook()

B, T, D, U = 32, 512, 512, 512
NCORES = 8
BL = B // NCORES          # 4 batches per core
KC = D // 128             # 4 contraction chunks (input proj)
UC = U // 128             # 4 contraction chunks (recurrent)
M_ALL = 3 * U // 128      # 12 output column chunks
SBLK = 32                 # steps per staged mx block
BODY = 2 * SBLK           # steps per For_i body (ping-pong A/B)

BF16 = mybir.dt.bfloat16
F32 = mybir.dt.float32
Alu = mybir.AluOpType
Act = mybir.ActivationFunctionType
ET = mybir.EngineType

_CACHE = {}
LAST_RESULT = None


def _build(T=T):
    nc = bacc.Bacc()
    xT = nc.declare_dram_parameter("xT", [D, BL * T], BF16, isOutput=False)
    wk = nc.declare_dram_parameter("wk", [D, 3 * U], BF16, isOutput=False)
    wr = nc.declare_dram_parameter("wr", [U, 3 * U], BF16, isOutput=False)
    bp = nc.declare_dram_parameter("bp", [3 * U], F32, isOutput=False)
    # out[u%128, u//128, t, b] (bf16; host upcasts)
    out = nc.declare_dram_parameter("out", [128, UC, T, BL], BF16, isOutput=True)

    with tile.TileContext(nc) as tc, ExitStack() as ctx:
        consts = ctx.enter_context(tc.tile_pool(name="consts", bufs=1))
        psum_p = ctx.enter_context(tc.tile_pool(name="psum", bufs=2, space="PSUM"))
        psum_1 = ctx.enter_context(tc.tile_pool(name="psum1", bufs=1, space="PSUM"))
        work = ctx.enter_context(tc.tile_pool(name="work", bufs=2))

        wk_sb = consts.tile([128, KC, 3 * U], BF16)
        nc.sync.dma_start(out=wk_sb, in_=wk.rearrange("(c p) n -> p c n", p=128))
        wr_sb = consts.tile([128, UC, 3 * U], BF16)
        nc.sync.dma_start(out=wr_sb, in_=wr.rearrange("(c p) n -> p c n", p=128))
        bp_sb = consts.tile([128, M_ALL], F32)
        nc.sync.dma_start(out=bp_sb, in_=bp.rearrange("(m p) -> p m", p=128))
        xT_sb = consts.tile([128, KC, BL * T], BF16)
        nc.sync.dma_start(out=xT_sb, in_=xT.rearrange("(c p) n -> p c n", p=128))
        ident = consts.tile([128, 128], BF16)
        make_identity(nc, ident)

        # mx^T [n%128, n//128, t, b] bf16, padded by BODY junk steps so the
        # ping-pong prefetch can always read a full block
        mx_sb = consts.tile([128, M_ALL, T + BODY, BL], BF16)
        nc.vector.memset(mx_sb[:, :, T:, :], 0.0)

        # ---- phase 1: mx^T = kernel^T @ x^T (+ bias', x0.2 pre-folded) ----
        # t-block-major so the first recurrence block's mx is ready after
        # 1/4 of phase1 (the rest overlaps the recurrence).
        xT_bt = xT_sb.rearrange("p c (b t) -> p c b t", b=BL)
        TB = T // 128
        for tb in range(TB):
            for m in range(M_ALL):
                ps = psum_p.tile([128, T], F32, tag="p1")
                for d in range(KC):
                    nc.tensor.matmul(
                        ps,
                        lhsT=wk_sb[:, d, m * 128:(m + 1) * 128],
                        rhs=xT_bt[:, d, :, tb * 128:(tb + 1) * 128],
                        start=(d == 0),
                        stop=(d == KC - 1),
                    )
                # psum free order is (b, t); reorder the mx view to match
                nc.scalar.activation(
                    out=mx_sb[:, m, tb * 128:(tb + 1) * 128, :].rearrange(
                        "p t b -> p b t"),
                    in_=ps, func=Act.Identity,
                    bias=bp_sb[:, m:m + 1],
                )

        # ---- phase 2: recurrence ----
        # persistent bf16 history: step s reads slot s, writes slot s+1;
        # the last step wraps to slot 0 (becomes next body's h_in) so no
        # carry copy is needed.
        hist = consts.tile([128, UC, BODY, BL], BF16)
        nc.vector.memset(hist[:, :, 0:1, :], 0.0)
        stgA = consts.tile([128, M_ALL, SBLK, BL], BF16)
        stgB = consts.tile([128, M_ALL, SBLK, BL], BF16)
        nc.sync.dma_start(out=stgA, in_=mx_sb[:, :, 0:SBLK, :])

        def step(stg, s, slot):
            out_slot = (slot + 1) % BODY
            h_in = hist[:, :, slot, :]                    # [128, UC, BL] bf16
            h_in4 = hist[:, :, slot:slot + 1, :]          # [128, UC, 1, BL]
            # Separate PSUM tiles for r / z / hh-halves so each consumer
            # depends only on its own accumulation group. One group per tile:
            # start on its first id-MM, stop on its last weight-MM. All id-MMs
            # (mx init, no h dependency) are emitted first so the PE runs them
            # during the previous step's blend.
            pr = psum_p.tile([128, 4, 1, BL], F32, tag="r")
            pzz = psum_1.tile([128, 4, 1, BL], F32, tag="z")
            phA = psum_1.tile([128, 2, 1, BL], F32, tag="hhA")
            phB = psum_1.tile([128, 2, 1, BL], F32, tag="hhB")
            for m in range(4):
                nc.tensor.matmul(
                    pr[:, m, 0, :], lhsT=ident, rhs=stg[:, 4 + m, s, :],
                    start=(m == 0), stop=False, skip_group_check=True,
                )
            for m in range(4):
                nc.tensor.matmul(
                    pzz[:, m, 0, :], lhsT=ident, rhs=stg[:, m, s, :],
                    start=(m == 0), stop=False, skip_group_check=True,
                )
            for m in range(2):
                nc.tensor.matmul(
                    phA[:, m, 0, :], lhsT=ident, rhs=stg[:, 8 + m, s, :],
                    start=(m == 0), stop=False, skip_group_check=True,
                )
            for m in range(2):
                nc.tensor.matmul(
                    phB[:, m, 0, :], lhsT=ident, rhs=stg[:, 10 + m, s, :],
                    start=(m == 0), stop=False, skip_group_check=True,
                )
            # r-gate weight MMs first, k-outer so the k=0,1 MMs only need the
            # first half of the blended h (chunked handoff from prev step)
            for k in range(UC):
                for m in range(4):
                    nc.tensor.matmul(
                        pr[:, m, 0, :],
                        lhsT=wr_sb[:, k, (4 + m) * 128:(5 + m) * 128],
                        rhs=h_in[:, k, :],
                        start=False,
                        stop=(k == UC - 1 and m == 3),
                        skip_group_check=True,
                    )
            for k in range(UC):
                for m in range(4):
                    nc.tensor.matmul(
                        pzz[:, m, 0, :],
                        lhsT=wr_sb[:, k, m * 128:(m + 1) * 128],
                        rhs=h_in[:, k, :],
                        start=False,
                        stop=(k == UC - 1 and m == 3),
                        skip_group_check=True,
                    )
            # r = clip(psum_r, 0, 1); rh = r * h   (unblocks hh matmuls)
            r_bf = work.tile([128, 4, 1, BL], BF16, tag="rbf")
            nc.vector.tensor_scalar(r_bf, pr, 1.0, 0.0,
                                    op0=Alu.min, op1=Alu.max)
            rh = work.tile([128, UC, 1, BL], BF16, tag="rh")
            rh_i = nc.vector.tensor_mul(rh, r_bf, h_in4)
            # hh pre-activation: psum = mx_h' + rh @ W_h; m-halves so tanh_A
            # can run while the B-half matmuls still execute
            for m in range(2):
                for k in range(UC):
                    nc.tensor.matmul(
                        phA[:, m, 0, :],
                        lhsT=wr_sb[:, k, 2 * U + m * 128:2 * U + (m + 1) * 128],
                        rhs=rh[:, k, 0, :],
                        start=False,
                        stop=(m == 1 and k == UC - 1),
                        skip_group_check=True,
                    )
            for m in range(2, 4):
                for k in range(UC):
                    nc.tensor.matmul(
                        phB[:, m - 2, 0, :],
                        lhsT=wr_sb[:, k, 2 * U + m * 128:2 * U + (m + 1) * 128],
                        rhs=rh[:, k, 0, :],
                        start=False,
                        stop=(m == 3 and k == UC - 1),
                        skip_group_check=True,
                    )
            # z ops off the critical chain (clip_z ordered after rh)
            z_bf = work.tile([128, 4, 1, BL], BF16, tag="zbf")
            zb_i = nc.vector.tensor_scalar(z_bf, pzz, 1.0, 0.0,
                                           op0=Alu.min, op1=Alu.max)
            add_dep_helper(zb_i.ins, rh_i.ins, sync=False,
                           reason="DVE critical chain first")
            w_t = work.tile([128, 4, 1, BL], F32, tag="wt")
            nc.vector.tensor_scalar(w_t, z_bf, -1.0, 1.0,
                                    op0=Alu.mult, op1=Alu.add)      # 1-z
            a_t = work.tile([128, 4, 1, BL], F32, tag="at")
            nc.vector.tensor_mul(a_t, z_bf, h_in4)
            # hh = tanh(psum); h' = (1-z)*hh + z*h, in halves -> hist out_slot
            hh_A = work.tile([128, 2, 1, BL], F32, tag="hhA2")
            nc.scalar.activation(out=hh_A, in_=phA, func=Act.Tanh)
            f_A = work.tile([128, 2, 1, BL], F32, tag="ftA")
            nc.vector.tensor_mul(f_A, w_t[:, 0:2, :, :], hh_A)
            nc.vector.tensor_add(hist[:, 0:2, out_slot:out_slot + 1, :],
                                 f_A, a_t[:, 0:2, :, :])
            hh_B = work.tile([128, 2, 1, BL], F32, tag="hhB2")
            nc.scalar.activation(out=hh_B, in_=phB, func=Act.Tanh)
            f_B = work.tile([128, 2, 1, BL], F32, tag="ftB")
            nc.vector.tensor_mul(f_B, w_t[:, 2:4, :, :], hh_B)
            nc.vector.tensor_add(hist[:, 2:4, out_slot:out_slot + 1, :],
                                 f_B, a_t[:, 2:4, :, :])

        with tc.For_i(0, T, BODY,
                      hint_engines=(ET.PE, ET.DVE, ET.Activation,
                                    ET.SP, ET.Pool)) as i:
            nc.sync.dma_start(out=stgB,
                              in_=mx_sb[:, :, bass.ds(i + SBLK, SBLK), :])
            for s in range(SBLK):
                step(stgA, s, s)
            nc.sync.dma_start(out=stgA,
                              in_=mx_sb[:, :, bass.ds(i + BODY, SBLK), :])
            for s in range(SBLK):
                step(stgB, s, SBLK + s)
            nc.sync.dma_start(out=out[:, :, bass.ds(i, BODY - 1), :],
                              in_=hist[:, :, 1:BODY, :])
            nc.sync.dma_start(out=out[:, :, bass.ds(i + BODY - 1, 1), :],
                              in_=hist[:, :, 0:1, :])
    return nc


def _graph():
    if "nc" not in _CACHE:
        nc = _build()
        if not nc.is_finalized():
            nc.finalize()
        _CACHE["nc"] = nc
    return _CACHE["nc"]


def kernel(x, kernel, recurrent_kernel, bias):
    global LAST_RESULT
    x = np.asarray(x, dtype=np.float32)
    wk_f = np.asarray(kernel, dtype=np.float32)
    wr_f = np.asarray(recurrent_kernel, dtype=np.float32)
    b_f = np.asarray(bias, dtype=np.float32)

    # fold hard_sigmoid affine (0.2*x + 0.5) into the z|r weight columns/bias
    scale = np.ones((3 * U,), np.float32)
    scale[: 2 * U] = 0.2
    wk_h = (wk_f * scale).astype(ml_dtypes.bfloat16)
    wr_h = (wr_f * scale).astype(ml_dtypes.bfloat16)
    bp_h = np.where(np.arange(3 * U) < 2 * U, 0.2 * b_f + 0.5, b_f).astype(np.float32)

    in_maps = []
    for c in range(NCORES):
        xs = x[c * BL:(c + 1) * BL]                       # [BL, T, D]
        xTc = np.ascontiguousarray(
            xs.transpose(2, 0, 1).reshape(D, BL * T)
        ).astype(ml_dtypes.bfloat16)
        in_maps.append({"xT": xTc, "wk": wk_h, "wr": wr_h, "bp": bp_h})

    res = run_bass_kernel_spmd(
        _graph(), in_maps, core_ids=list(range(NCORES)),
        trace=bool(os.environ.get("GRU_TRACE")),
    )
    LAST_RESULT = res

    outs = []
    for c in range(NCORES):
        arr = np.asarray(res.results[c]["out"]).astype(np.float32)
        outs.append(np.transpose(arr, (3, 2, 1, 0)).reshape(BL, T, U))
    return np.concatenate(outs, axis=0)
